# revision 4
# baseline (speedup 1.0000x reference)
"""Cross-attention Bass kernel for Trainium2, 8 NeuronCores, head-sharded.

Reference semantics: q = RMSNorm_head(x@Wq.T+bq), kv = c@Wkv.T+bkv (k/v
interleaved), k = RMSNorm_head(k), out = softmax(q k^T/sqrt(dh)) v, merged
heads -> [b, n, dim].

Sharding: 16 heads over 8 cores (2 heads each). Each core reads full x, c
(pre-transposed and cast to bf16 on the host) and its weight slices; ships
un-normalized U.T tiles (with the softmax denominator in row 64) back to the
host, which does the final divide / bias-add / transpose in fp32.

Per-core pipeline (bf16 matmuls, fp32 PSUM):
  Projection phase (per 512-row chunk of xT/cT, 8 chunks over both batches):
    - plain DMA loads of xT/cT tiles [128k, 512seq] (host pre-transposed, so
      no on-device transposes of the activations)
    - W-stationary projections -> qT/kT/vT [head_dims, seq] in PSUM
    - RMSNorm in T layout: per-head sumsq via indicator-matmul (1/gamma^2
      folded for k; gamma_q*gamma_k folded into Wk/bk on host), sqrt on ACT,
      reciprocal on DVE, partition-broadcast with a K=2 expander matmul.
    - V transposed to natural [m, dh] (PE, 4 tiles/chunk) with a ones column
      at position 64 so the softmax denominator rides the AV matmul. V's bias
      is NOT applied on device: softmax(P) @ (V0+bv) = P@V0/den + bv, so the
      host adds bv after the division.
  Attention phase (per batch, per head, m-tile outer over full n=2048):
    S.T[m,:] = kt_tile.T @ qt (4 N=512 matmuls from one stationary load),
    exp(S.T/8) on ACT in one [128,2048] op (PSUM->SBUF bf16),
    U.T[nh] += V'.T @ expS.T into 4 PSUM accumulation banks.
    After 16 m-tiles: copy U.T rows 0..64 (64 dims + denominator) to SBUF
    bf16 and DMA to DRAM; host finishes.

Phases are separated (not interleaved) so attention can use all 8 PSUM banks;
the exp() on ACT overlaps the S/U matmuls on PE within the m-tile pipeline.
"""

import sys

sys.path.insert(0, "/opt/trn_rl_repo")

import numpy as np
import ml_dtypes
from contextlib import ExitStack

import concourse.bass as bass
import concourse.tile as tile
from concourse import bacc, mybir
from concourse.bass_utils import run_bass_kernel_spmd
from concourse.masks import make_identity

F32 = mybir.dt.float32
F32R = mybir.dt.float32r
BF16 = mybir.dt.bfloat16
NP_BF16 = ml_dtypes.bfloat16

DIM = 1024
H = 16
DH = 64
B = 2
N = 2048
ROWS = B * N            # 4096 flattened rows
NC = 8
HPC = H // NC           # 2 heads per core
EPS = 1.1920928955078125e-07

NKB = DIM // 128        # 8 k-tiles
CPB = N // 512          # 4 chunks of 512 rows per batch
NCH = ROWS // 512       # 8 chunks total
MT_PER_B = N // 128     # 16 m-tiles per batch
NH = N // 512           # 4 n-slices of 512 per batch

LAST_EXEC_TIME_NS = None
LAST_RESULTS = None
_LAST_IN_MAPS = None


class _Ctx:
    pass


def build_bass(dbg=False, reps=1):
    nc = bacc.Bacc("TRN2", target_bir_lowering=False, debug=False)
    g = _Ctx()
    g.nc = nc

    g.xT = nc.dram_tensor("xT", [DIM, ROWS], BF16, kind="ExternalInput")
    g.cT = nc.dram_tensor("cT", [DIM, ROWS], BF16, kind="ExternalInput")
    g.wq = nc.dram_tensor("wq", [DIM, 128], BF16, kind="ExternalInput")
    g.wk = nc.dram_tensor("wk", [DIM, 128], BF16, kind="ExternalInput")
    g.wv = nc.dram_tensor("wv", [DIM, 128], BF16, kind="ExternalInput")
    g.bq_d = nc.dram_tensor("bq", [128, 1], F32, kind="ExternalInput")
    g.bk_d = nc.dram_tensor("bk", [128, 1], F32, kind="ExternalInput")
    g.gq_d = nc.dram_tensor("gq", [128, 2], F32R, kind="ExternalInput")
    g.gk_d = nc.dram_tensor("gk", [128, 2], F32R, kind="ExternalInput")
    # U.T tiles: rows 0..63 = sum_m P*V0, row 64 = softmax denominator.
    # One [65, 2048] tile per (batch, head), packed along the free dim.
    g.uout = nc.dram_tensor("uout", [65, B * HPC * N], BF16,
                            kind="ExternalOutput")

    with tile.TileContext(nc) as tc, ExitStack() as ctx:
        g.tc = tc
        const = ctx.enter_context(tc.tile_pool(name="const", bufs=1))
        resid = ctx.enter_context(tc.tile_pool(name="resid", bufs=1))
        g.ld = ctx.enter_context(tc.tile_pool(name="ld", bufs=4))
        g.tmp = ctx.enter_context(tc.tile_pool(name="tmpA", bufs=2))
        g.small = ctx.enter_context(tc.tile_pool(name="small", bufs=2))
        g.esb = ctx.enter_context(tc.tile_pool(name="esb", bufs=3))
        g.usb = ctx.enter_context(tc.tile_pool(name="usb", bufs=2))

        ident_f = const.tile([128, 128], F32, tag="identf")
        make_identity(nc, ident_f[:])
        g.ident = const.tile([128, 128], F32R, tag="ident")
        nc.vector.tensor_copy(g.ident[:], ident_f[:])

        g.wq_sb = const.tile([128, NKB, 128], BF16, tag="wq")
        g.wk_sb = const.tile([128, NKB, 128], BF16, tag="wk")
        g.wv_sb = const.tile([128, NKB, 128], BF16, tag="wv")
        for kb in range(NKB):
            nc.sync.dma_start(g.wq_sb[:, kb], g.wq[kb * 128:(kb + 1) * 128, :])
            nc.sync.dma_start(g.wk_sb[:, kb], g.wk[kb * 128:(kb + 1) * 128, :])
            nc.sync.dma_start(g.wv_sb[:, kb], g.wv[kb * 128:(kb + 1) * 128, :])
        g.bq_sb = const.tile([128, 1], F32, tag="bq")
        g.bk_sb = const.tile([128, 1], F32, tag="bk")
        g.gq_sb = const.tile([128, 2], F32R, tag="gq")
        g.gk_sb = const.tile([128, 2], F32R, tag="gk")
        nc.sync.dma_start(g.bq_sb[:], g.bq_d[:])
        nc.sync.dma_start(g.bk_sb[:], g.bk_d[:])
        nc.sync.dma_start(g.gq_sb[:], g.gq_d[:])
        nc.sync.dma_start(g.gk_sb[:], g.gk_d[:])
        g.eps_sb = const.tile([128, 1], F32, tag="eps")
        nc.gpsimd.memset(g.eps_sb[:], EPS)

        # expander: expand[x, y] = 1 iff y//64 == x  (rb[p] = rinv[p//64])
        expand_f = const.tile([2, 128], F32, tag="expand_f")
        nc.gpsimd.memset(expand_f[:], 0.0)
        nc.gpsimd.affine_select(
            out=expand_f[:], in_=expand_f[:],
            compare_op=mybir.AluOpType.is_ge, fill=1.0,
            base=-64, pattern=[[1, 128]], channel_multiplier=-64)
        nc.gpsimd.affine_select(
            out=expand_f[:], in_=expand_f[:],
            compare_op=mybir.AluOpType.is_ge, fill=0.0,
            base=0, pattern=[[1, 128]], channel_multiplier=-64)
        g.expand_b = const.tile([2, 128], F32R, tag="expand_b")
        nc.vector.tensor_copy(g.expand_b[:], expand_f[:])

        ones64 = const.tile([128, 64], F32, tag="ones64")
        nc.gpsimd.memset(ones64[:], 1.0)
        z1 = const.tile([128, 1], F32, tag="z1")
        nc.gpsimd.memset(z1[:], 0.0)

        # per-batch residents (bf16)
        g.qt = [resid.tile([128, N], BF16, tag=f"qt{b}", name=f"qt{b}")
                for b in range(B)]
        g.kt = [resid.tile([128, N], BF16, tag=f"kt{b}", name=f"kt{b}")
                for b in range(B)]
        g.v2 = [resid.tile([128, MT_PER_B, 2, 128], BF16, tag=f"v2{b}",
                           name=f"v2{b}")
                for b in range(B)]
        for b in range(B):
            nc.vector.tensor_copy(
                g.v2[b][:, :, :, 64:65],
                ones64[:, 0:MT_PER_B * 2].rearrange(
                    "p (a b c) -> p a b c", a=MT_PER_B, b=2))
            nc.vector.tensor_copy(
                g.v2[b][:, :, :, 65:128],
                z1[:].broadcast_to((128, MT_PER_B, 2, 63)))

        for _ in range(reps):
            # all projections first (both batches), then attention; the
            # phases are separated so attention can use all 8 PSUM banks
            with tc.tile_pool(name="ppsum", bufs=3, space="PSUM") as pp, \
                    tc.tile_pool(name="npsum", bufs=2, space="PSUM") as pn, \
                    tc.tile_pool(name="vpsum", bufs=2, space="PSUM") as pv:
                g.pp, g.pn, g.pv = pp, pn, pv
                for ch in range(NCH):
                    _proj_chunk(g, ch)
            with tc.tile_pool(name="spsum", bufs=1, space="PSUM") as ps, \
                    tc.tile_pool(name="upsum", bufs=1, space="PSUM") as pu:
                g.ps, g.pu = ps, pu
                for b in range(B):
                    for h in range(HPC):
                        _attn_head(g, b, h)

        if dbg:
            qt_d = nc.dram_tensor("qt_dbg", [128, ROWS], F32,
                                  kind="ExternalOutput")
            kt_d = nc.dram_tensor("kt_dbg", [128, ROWS], F32,
                                  kind="ExternalOutput")
            v2_d = nc.dram_tensor("v2_dbg", [128, ROWS * 2], F32,
                                  kind="ExternalOutput")
            for b in range(B):
                qf = g.usb.tile([128, N], F32, tag="dbgq")
                nc.vector.tensor_copy(qf[:], g.qt[b][:])
                nc.sync.dma_start(qt_d[:, b * N:(b + 1) * N], qf[:])
                kf = g.usb.tile([128, N], F32, tag="dbgk")
                nc.vector.tensor_copy(kf[:], g.kt[b][:])
                nc.sync.dma_start(kt_d[:, b * N:(b + 1) * N], kf[:])
                vf = g.usb.tile([128, N * 2], F32, tag="dbgv")
                nc.vector.tensor_copy(
                    vf[:], g.v2[b][:].rearrange("p a b e -> p (a b e)"))
                nc.sync.dma_start(v2_d[:, b * N * 2:(b + 1) * N * 2], vf[:])

    nc.compile()
    return nc


def _norm_T(g, lin_ps, bias_sb, g_sb, dst_ap):
    """RMSNorm in T layout: dst = (lin+bias) * rsqrt(mean(sq)+eps) per head."""
    nc = g.nc
    s_sb = g.tmp.tile([128, 512], F32, tag="lin")
    nc.vector.tensor_scalar_add(s_sb[:], lin_ps[:], bias_sb[:])
    sq = g.tmp.tile([128, 512], F32R, tag="sq")
    nc.vector.tensor_tensor(
        out=sq[:], in0=s_sb[:], in1=s_sb[:], op=mybir.AluOpType.mult)
    ss = g.pn.tile([128, 512], F32, tag="nrm", name="ss")
    nc.tensor.matmul(ss[0:2, :], g_sb[:], sq[:])
    rms = g.small.tile([2, 512], F32, tag="rms")
    nc.scalar.activation(
        rms[:], ss[0:2, :], mybir.ActivationFunctionType.Sqrt,
        bias=g.eps_sb[0:2, :], scale=1.0 / DH)
    rinv = g.small.tile([2, 512], F32R, tag="rinv")
    with nc.allow_low_precision(reason="rsqrt scale fed to bf16 pipeline"):
        nc.vector.reciprocal(rinv[:], rms[:])
    rb = g.pn.tile([128, 512], F32, tag="nrm", name="rb")
    nc.tensor.matmul(rb[:], g.expand_b[:], rinv[:])
    nc.vector.tensor_tensor(
        out=dst_ap, in0=s_sb[:], in1=rb[:], op=mybir.AluOpType.mult)


def _proj_chunk(g, ch):
    nc = g.nc
    b = ch // CPB
    cols = bass.ds((ch % CPB) * 512, 512)
    dcols = bass.ds(ch * 512, 512)

    xt = g.ld.tile([128, NKB, 512], BF16, tag="xt")
    for kb in range(NKB):
        nc.sync.dma_start(xt[:, kb], g.xT[kb * 128:(kb + 1) * 128, dcols])
    ct = g.ld.tile([128, NKB, 512], BF16, tag="ct")
    for kb in range(NKB):
        nc.sync.dma_start(ct[:, kb], g.cT[kb * 128:(kb + 1) * 128, dcols])

    q_ps = g.pp.tile([128, 512], F32, tag="qkv", name="q_ps")
    for kb in range(NKB):
        nc.tensor.matmul(q_ps[:], g.wq_sb[:, kb], xt[:, kb],
                         start=(kb == 0), stop=(kb == NKB - 1))
    _norm_T(g, q_ps, g.bq_sb, g.gq_sb, g.qt[b][:, cols])

    k_ps = g.pp.tile([128, 512], F32, tag="qkv", name="k_ps")
    for kb in range(NKB):
        nc.tensor.matmul(k_ps[:], g.wk_sb[:, kb], ct[:, kb],
                         start=(kb == 0), stop=(kb == NKB - 1))
    _norm_T(g, k_ps, g.bk_sb, g.gk_sb, g.kt[b][:, cols])

    v_ps = g.pp.tile([128, 512], F32, tag="qkv", name="v_ps")
    for kb in range(NKB):
        nc.tensor.matmul(v_ps[:], g.wv_sb[:, kb], ct[:, kb],
                         start=(kb == 0), stop=(kb == NKB - 1))
    v_sb = g.tmp.tile([128, 512], F32R, tag="vsb")
    nc.vector.tensor_copy(v_sb[:], v_ps[:])
    vn = g.pv.tile([128, 512], F32, tag="vn")
    for t in range(4):
        nc.tensor.transpose(
            vn[:, t * 128:(t + 1) * 128].bitcast(F32R),
            v_sb[:, t * 128:(t + 1) * 128],
            g.ident[:])
    mt0 = (ch % CPB) * 4
    nc.vector.tensor_copy(
        g.v2[b][:, mt0:mt0 + 4, :, 0:64],
        vn[:].rearrange("p (t h e) -> p t h e", t=4, h=2))


def _attn_head(g, b, h):
    nc = g.nc
    qt, kt, v2 = g.qt[b], g.kt[b], g.v2[b]
    hp = bass.ds(h * 64, 64)
    u = [g.pu.tile([128, 512], F32, tag=f"u{j}", name=f"u{j}")
         for j in range(NH)]
    for mt in range(MT_PER_B):
        mcols = bass.ds(mt * 128, 128)
        s_ps = g.ps.tile([128, N], F32, tag="s")
        for nh in range(NH):
            ncols = bass.ds(nh * 512, 512)
            nc.tensor.matmul(s_ps[:, ncols], kt[hp, mcols], qt[hp, ncols])
        e_sb = g.esb.tile([128, N], BF16, tag="e")
        nc.scalar.activation(
            e_sb[:], s_ps[:], mybir.ActivationFunctionType.Exp, scale=0.125)
        for nh in range(NH):
            ncols = bass.ds(nh * 512, 512)
            nc.tensor.matmul(u[nh][:], v2[:, mt, h], e_sb[:, ncols],
                             start=(mt == 0), stop=(mt == MT_PER_B - 1),
                             skip_group_check=True)
    u_sb = g.usb.tile([65, N], BF16, tag="usb")
    for nh in range(NH):
        ncols = bass.ds(nh * 512, 512)
        nc.vector.tensor_copy(u_sb[:, ncols], u[nh][0:65, :])
    off = (b * HPC + h) * N
    nc.sync.dma_start(g.uout[:, off:off + N], u_sb[:])


_CACHED_NC = None


def kernel(x, c, Wq, bq, Wkv, bkv, q_gamma, k_gamma, _trace=False, _dbg=False):
    global LAST_EXEC_TIME_NS, LAST_RESULTS, _CACHED_NC, _LAST_IN_MAPS

    x = np.asarray(x, dtype=np.float32)
    c = np.asarray(c, dtype=np.float32)
    Wq = np.asarray(Wq, dtype=np.float32)
    bq = np.asarray(bq, dtype=np.float32)
    Wkv = np.asarray(Wkv, dtype=np.float32)
    bkv = np.asarray(bkv, dtype=np.float32)
    q_gamma = np.asarray(q_gamma, dtype=np.float32)
    k_gamma = np.asarray(k_gamma, dtype=np.float32)

    b, n, _ = x.shape
    xT = np.ascontiguousarray(x.reshape(ROWS, DIM).T.astype(NP_BF16))
    cT = np.ascontiguousarray(c.reshape(ROWS, DIM).T.astype(NP_BF16))

    g2 = q_gamma * k_gamma                      # [64]
    g2_2 = np.tile(g2, HPC)                     # [128]
    d2 = np.arange(DH)

    in_maps = []
    bv_locals = []
    for i in range(NC):
        h0 = i * HPC
        rows_q = np.concatenate(
            [h * DH + d2 for h in range(h0, h0 + HPC)])
        k_rows = np.concatenate(
            [h * 2 * DH + 2 * d2 for h in range(h0, h0 + HPC)])
        v_rows = k_rows + 1

        wq_t = np.ascontiguousarray(Wq[rows_q].T.astype(NP_BF16))
        wk_t = np.ascontiguousarray(
            (Wkv[k_rows] * g2_2[:, None]).T.astype(NP_BF16))
        wv_t = np.ascontiguousarray(Wkv[v_rows].T.astype(NP_BF16))
        bq_l = np.ascontiguousarray(bq[rows_q].reshape(128, 1))
        bk_l = np.ascontiguousarray((bkv[k_rows] * g2_2).reshape(128, 1))
        bv_locals.append(bkv[v_rows])

        gq_l = np.zeros((128, 2), dtype=np.float32)
        gk_l = np.zeros((128, 2), dtype=np.float32)
        for h in range(HPC):
            gq_l[h * DH:(h + 1) * DH, h] = 1.0
            gk_l[h * DH:(h + 1) * DH, h] = 1.0 / (g2 * g2)
        in_maps.append({
            "xT": xT, "cT": cT,
            "wq": wq_t, "wk": wk_t, "wv": wv_t,
            "bq": bq_l, "bk": bk_l,
            "gq": gq_l, "gk": gk_l,
        })

    _LAST_IN_MAPS = in_maps
    if _CACHED_NC is None:
        _CACHED_NC = build_bass(dbg=_dbg)
    nc = _CACHED_NC

    res = run_bass_kernel_spmd(
        nc, in_maps, core_ids=list(range(NC)), trace=_trace)
    LAST_EXEC_TIME_NS = res.exec_time_ns
    LAST_RESULTS = res

    # host-side finish: divide by the denominator (row 64), add bv, transpose
    full = np.empty((b, n, DIM), dtype=np.float32)
    for i in range(NC):
        u = np.asarray(res.results[i]["uout"], dtype=np.float32)  # [65, 8192]
        bv_l = bv_locals[i]
        for bb in range(B):
            for h in range(HPC):
                s = u[:, (bb * HPC + h) * N:(bb * HPC + h + 1) * N]
                o = (s[0:64] / s[64:65]).T + bv_l[h * DH:(h + 1) * DH]
                full[bb, :, i * 128 + h * DH: i * 128 + (h + 1) * DH] = o
    return full


# revision 7
# speedup vs baseline: 1.4389x; 1.4389x over previous
"""Cross-attention Bass kernel for Trainium2, 8 NeuronCores, head-sharded.

Reference semantics: q = RMSNorm_head(x@Wq.T+bq), kv = c@Wkv.T+bkv (k/v
interleaved), k = RMSNorm_head(k), out = softmax(q k^T/sqrt(dh)) v, merged
heads -> [b, n, dim].

Sharding: 16 heads over 8 cores (2 heads each). Each core reads full x, c
(pre-transposed and cast to bf16 on the host) and its weight slices; ships
un-normalized U.T tiles (with the softmax denominator in row 64) back to the
host, which does the final divide / bias-add / transpose in fp32.

Per-core pipeline (bf16 matmuls, fp32 PSUM):
  Projection phase (per 512-row chunk of xT/cT, 8 chunks over both batches):
    - plain DMA loads of xT/cT tiles [128k, 512seq] (host pre-transposed, so
      no on-device transposes of the activations)
    - W-stationary projections -> qT/kT/vT [head_dims, seq] in PSUM
    - RMSNorm in T layout: per-head sumsq via indicator-matmul (1/gamma^2
      folded for k; gamma_q*gamma_k folded into Wk/bk on host), sqrt on ACT,
      reciprocal on DVE, partition-broadcast with a K=2 expander matmul.
    - V transposed to natural [m, dh] (PE, 4 tiles/chunk) with a ones column
      at position 64 so the softmax denominator rides the AV matmul. V's bias
      is NOT applied on device: softmax(P) @ (V0+bv) = P@V0/den + bv, so the
      host adds bv after the division.
  Attention phase (per batch, per head, m-tile outer over full n=2048):
    S.T[m,:] = kt_tile.T @ qt (4 N=512 matmuls from one stationary load),
    exp(S.T/8) on ACT in one [128,2048] op (PSUM->SBUF bf16),
    U.T[nh] += V'.T @ expS.T into 4 PSUM accumulation banks.
    After 16 m-tiles: copy U.T rows 0..64 (64 dims + denominator) to SBUF
    bf16 and DMA to DRAM; host finishes.

Phases are separated (not interleaved) so attention can use all 8 PSUM banks;
the exp() on ACT overlaps the S/U matmuls on PE within the m-tile pipeline.
"""

import sys

sys.path.insert(0, "/opt/trn_rl_repo")

import numpy as np
import ml_dtypes
from contextlib import ExitStack

import concourse.bass as bass
import concourse.tile as tile
from concourse import bacc, mybir
from concourse.bass_utils import run_bass_kernel_spmd
from concourse.masks import make_identity

F32 = mybir.dt.float32
F32R = mybir.dt.float32r
BF16 = mybir.dt.bfloat16
NP_BF16 = ml_dtypes.bfloat16

DIM = 1024
H = 16
DH = 64
B = 2
N = 2048
ROWS = B * N            # 4096 flattened rows
NC = 8
HPC = H // NC           # 2 heads per core
EPS = 1.1920928955078125e-07

NKB = DIM // 128        # 8 k-tiles
CPB = N // 512          # 4 chunks of 512 rows per batch
NCH = ROWS // 512       # 8 chunks total
MT_PER_B = N // 128     # 16 m-tiles per batch
NH = N // 512           # 4 n-slices of 512 per batch

LAST_EXEC_TIME_NS = None
LAST_RESULTS = None
_LAST_IN_MAPS = None


class _Ctx:
    pass


def build_bass(dbg=False, reps=1):
    nc = bacc.Bacc("TRN2", target_bir_lowering=False, debug=False)
    g = _Ctx()
    g.nc = nc

    g.xT = nc.dram_tensor("xT", [DIM, ROWS], BF16, kind="ExternalInput")
    g.cT = nc.dram_tensor("cT", [DIM, ROWS], BF16, kind="ExternalInput")
    g.wq = nc.dram_tensor("wq", [DIM, 128], BF16, kind="ExternalInput")
    g.wk = nc.dram_tensor("wk", [DIM, 128], BF16, kind="ExternalInput")
    g.wv = nc.dram_tensor("wv", [DIM, 128], BF16, kind="ExternalInput")
    g.bq_d = nc.dram_tensor("bq", [128, 1], F32, kind="ExternalInput")
    g.bk_d = nc.dram_tensor("bk", [128, 1], F32, kind="ExternalInput")
    g.gq_d = nc.dram_tensor("gq", [128, 2], F32R, kind="ExternalInput")
    g.gk_d = nc.dram_tensor("gk", [128, 2], F32R, kind="ExternalInput")
    # U.T tiles: rows 0..63 = sum_m P*V0, row 64 = softmax denominator.
    # One [65, 2048] tile per (batch, head), packed along the free dim.
    g.uout = nc.dram_tensor("uout", [65, B * HPC * N], BF16,
                            kind="ExternalOutput")

    with tile.TileContext(nc) as tc, ExitStack() as ctx:
        g.tc = tc
        const = ctx.enter_context(tc.tile_pool(name="const", bufs=1))
        resid = ctx.enter_context(tc.tile_pool(name="resid", bufs=1))
        g.ld = ctx.enter_context(tc.tile_pool(name="ld", bufs=4))
        g.tmp = ctx.enter_context(tc.tile_pool(name="tmpA", bufs=2))
        g.small = ctx.enter_context(tc.tile_pool(name="small", bufs=2))
        g.esb = ctx.enter_context(tc.tile_pool(name="esb", bufs=3))
        g.usb = ctx.enter_context(tc.tile_pool(name="usb", bufs=2))

        ident_f = const.tile([128, 128], F32, tag="identf")
        make_identity(nc, ident_f[:])
        g.ident = const.tile([128, 128], F32R, tag="ident")
        nc.vector.tensor_copy(g.ident[:], ident_f[:])

        g.wq_sb = const.tile([128, NKB, 128], BF16, tag="wq")
        g.wk_sb = const.tile([128, NKB, 128], BF16, tag="wk")
        g.wv_sb = const.tile([128, NKB, 128], BF16, tag="wv")
        for kb in range(NKB):
            nc.sync.dma_start(g.wq_sb[:, kb], g.wq[kb * 128:(kb + 1) * 128, :])
            nc.sync.dma_start(g.wk_sb[:, kb], g.wk[kb * 128:(kb + 1) * 128, :])
            nc.sync.dma_start(g.wv_sb[:, kb], g.wv[kb * 128:(kb + 1) * 128, :])
        g.bq_sb = const.tile([128, 1], F32, tag="bq")
        g.bk_sb = const.tile([128, 1], F32, tag="bk")
        g.gq_sb = const.tile([128, 2], F32R, tag="gq")
        g.gk_sb = const.tile([128, 2], F32R, tag="gk")
        nc.sync.dma_start(g.bq_sb[:], g.bq_d[:])
        nc.sync.dma_start(g.bk_sb[:], g.bk_d[:])
        nc.sync.dma_start(g.gq_sb[:], g.gq_d[:])
        nc.sync.dma_start(g.gk_sb[:], g.gk_d[:])
        g.eps_sb = const.tile([128, 1], F32, tag="eps")
        nc.gpsimd.memset(g.eps_sb[:], EPS)

        # expander: expand[x, y] = 1 iff y//64 == x  (rb[p] = rinv[p//64])
        expand_f = const.tile([2, 128], F32, tag="expand_f")
        nc.gpsimd.memset(expand_f[:], 0.0)
        nc.gpsimd.affine_select(
            out=expand_f[:], in_=expand_f[:],
            compare_op=mybir.AluOpType.is_ge, fill=1.0,
            base=-64, pattern=[[1, 128]], channel_multiplier=-64)
        nc.gpsimd.affine_select(
            out=expand_f[:], in_=expand_f[:],
            compare_op=mybir.AluOpType.is_ge, fill=0.0,
            base=0, pattern=[[1, 128]], channel_multiplier=-64)
        g.expand_b = const.tile([2, 128], F32R, tag="expand_b")
        nc.vector.tensor_copy(g.expand_b[:], expand_f[:])

        ones64 = const.tile([128, 64], F32, tag="ones64")
        nc.gpsimd.memset(ones64[:], 1.0)
        z1 = const.tile([128, 1], F32, tag="z1")
        nc.gpsimd.memset(z1[:], 0.0)

        # per-batch residents (bf16)
        g.qt = [resid.tile([128, N], BF16, tag=f"qt{b}", name=f"qt{b}")
                for b in range(B)]
        g.kt = [resid.tile([128, N], BF16, tag=f"kt{b}", name=f"kt{b}")
                for b in range(B)]
        g.v2 = [resid.tile([128, MT_PER_B, 2, 128], BF16, tag=f"v2{b}",
                           name=f"v2{b}")
                for b in range(B)]
        for b in range(B):
            nc.vector.tensor_copy(
                g.v2[b][:, :, :, 64:65],
                ones64[:, 0:MT_PER_B * 2].rearrange(
                    "p (a b c) -> p a b c", a=MT_PER_B, b=2))
            nc.vector.tensor_copy(
                g.v2[b][:, :, :, 65:128],
                z1[:].broadcast_to((128, MT_PER_B, 2, 63)))

        for _ in range(reps):
            # all projections first (both batches), then attention; the
            # phases are separated so attention can use all 8 PSUM banks
            with tc.tile_pool(name="ppsum", bufs=3, space="PSUM") as pp, \
                    tc.tile_pool(name="npsum", bufs=2, space="PSUM") as pn, \
                    tc.tile_pool(name="vpsum", bufs=2, space="PSUM") as pv:
                g.pp, g.pn, g.pv = pp, pn, pv
                for ch in range(NCH):
                    _proj_chunk(g, ch)
            with tc.tile_pool(name="spsum", bufs=2, space="PSUM") as ps, \
                    tc.tile_pool(name="upsum", bufs=1, space="PSUM") as pu:
                g.ps, g.pu = ps, pu
                for b in range(B):
                    for h in range(HPC):
                        _attn_head(g, b, h)

        if dbg:
            qt_d = nc.dram_tensor("qt_dbg", [128, ROWS], F32,
                                  kind="ExternalOutput")
            kt_d = nc.dram_tensor("kt_dbg", [128, ROWS], F32,
                                  kind="ExternalOutput")
            v2_d = nc.dram_tensor("v2_dbg", [128, ROWS * 2], F32,
                                  kind="ExternalOutput")
            for b in range(B):
                qf = g.usb.tile([128, N], F32, tag="dbgq")
                nc.vector.tensor_copy(qf[:], g.qt[b][:])
                nc.sync.dma_start(qt_d[:, b * N:(b + 1) * N], qf[:])
                kf = g.usb.tile([128, N], F32, tag="dbgk")
                nc.vector.tensor_copy(kf[:], g.kt[b][:])
                nc.sync.dma_start(kt_d[:, b * N:(b + 1) * N], kf[:])
                vf = g.usb.tile([128, N * 2], F32, tag="dbgv")
                nc.vector.tensor_copy(
                    vf[:], g.v2[b][:].rearrange("p a b e -> p (a b e)"))
                nc.sync.dma_start(v2_d[:, b * N * 2:(b + 1) * N * 2], vf[:])

    nc.compile()
    return nc


def _norm_pre(g, lin_ps, bias_sb, name):
    """bias-add + square (DVE); returns (s_sb, sq) for the sumsq matmul."""
    nc = g.nc
    s_sb = g.tmp.tile([128, 512], F32, tag="lin", name=f"lin_{name}")
    nc.vector.tensor_scalar_add(s_sb[:], lin_ps[:], bias_sb[:])
    sq = g.tmp.tile([128, 512], F32R, tag="sq", name=f"sq_{name}")
    nc.vector.tensor_tensor(
        out=sq[:], in0=s_sb[:], in1=s_sb[:], op=mybir.AluOpType.mult)
    return s_sb, sq


def _norm_ss(g, g_sb, sq, name):
    """per-head sumsq via indicator matmul -> [2, 512] PSUM."""
    nc = g.nc
    ss = g.pn.tile([128, 512], F32, tag="nrm", name=f"ss_{name}")
    nc.tensor.matmul(ss[0:2, :], g_sb[:], sq[:])
    return ss


def _norm_rinv(g, ss, name):
    """sqrt on ACT, then single-instruction approx reciprocal on DVE
    (nc.vector.reciprocal is ~6 cpe iterative-divide, 3.3us per [2,512];
    the approx variant is ~51 ULP which is far below the bf16 noise)."""
    nc = g.nc
    rms = g.small.tile([2, 512], F32, tag="rms", name=f"rms_{name}")
    nc.scalar.activation(
        rms[:], ss[0:2, :], mybir.ActivationFunctionType.Sqrt,
        bias=g.eps_sb[0:2, :], scale=1.0 / DH)
    rinv = g.small.tile([2, 512], F32, tag="rinv", name=f"rinv_{name}")
    nc.vector.reciprocal_approx_fast(rinv[:], rms[:])
    rinv_r = g.small.tile([2, 512], F32R, tag="rinvr", name=f"rinvr_{name}")
    nc.vector.tensor_copy(rinv_r[:], rinv[:])
    return rinv_r


def _norm_fin(g, s_sb, rinv, dst_ap, name):
    """broadcast rinv down partitions (K=2 expander matmul) and scale."""
    nc = g.nc
    rb = g.pn.tile([128, 512], F32, tag="nrm", name=f"rb_{name}")
    nc.tensor.matmul(rb[:], g.expand_b[:], rinv[:])
    nc.vector.tensor_tensor(
        out=dst_ap, in0=s_sb[:], in1=rb[:], op=mybir.AluOpType.mult)


def _proj_chunk(g, ch):
    # emission order keeps the PE stream dense: the norm matmuls (ss, rb)
    # land between the q/k/v projection groups so their DVE/ACT inputs have
    # had time to complete, and the PE never waits on the norm chain.
    nc = g.nc
    b = ch // CPB
    cols = bass.ds((ch % CPB) * 512, 512)
    dcols = bass.ds(ch * 512, 512)

    xt = g.ld.tile([128, NKB, 512], BF16, tag="xt")
    for kb in range(NKB):
        nc.sync.dma_start(xt[:, kb], g.xT[kb * 128:(kb + 1) * 128, dcols])
    ct = g.ld.tile([128, NKB, 512], BF16, tag="ct")
    for kb in range(NKB):
        nc.sync.dma_start(ct[:, kb], g.cT[kb * 128:(kb + 1) * 128, dcols])

    q_ps = g.pp.tile([128, 512], F32, tag="qkv", name="q_ps")
    for kb in range(NKB):
        nc.tensor.matmul(q_ps[:], g.wq_sb[:, kb], xt[:, kb],
                         start=(kb == 0), stop=(kb == NKB - 1))
    sq_s, sq_sq = _norm_pre(g, q_ps, g.bq_sb, "q")

    k_ps = g.pp.tile([128, 512], F32, tag="qkv", name="k_ps")
    for kb in range(NKB):
        nc.tensor.matmul(k_ps[:], g.wk_sb[:, kb], ct[:, kb],
                         start=(kb == 0), stop=(kb == NKB - 1))
    ss_q = _norm_ss(g, g.gq_sb, sq_sq, "q")
    rinv_q = _norm_rinv(g, ss_q, "q")
    sk_s, sk_sq = _norm_pre(g, k_ps, g.bk_sb, "k")

    v_ps = g.pp.tile([128, 512], F32, tag="qkv", name="v_ps")
    for kb in range(NKB):
        nc.tensor.matmul(v_ps[:], g.wv_sb[:, kb], ct[:, kb],
                         start=(kb == 0), stop=(kb == NKB - 1))
    ss_k = _norm_ss(g, g.gk_sb, sk_sq, "k")
    rinv_k = _norm_rinv(g, ss_k, "k")
    v_sb = g.tmp.tile([128, 512], F32R, tag="vsb")
    nc.vector.tensor_copy(v_sb[:], v_ps[:])

    _norm_fin(g, sq_s, rinv_q, g.qt[b][:, cols], "q")
    vn = g.pv.tile([128, 512], F32, tag="vn")
    for t in range(4):
        nc.tensor.transpose(
            vn[:, t * 128:(t + 1) * 128].bitcast(F32R),
            v_sb[:, t * 128:(t + 1) * 128],
            g.ident[:])
    _norm_fin(g, sk_s, rinv_k, g.kt[b][:, cols], "k")
    mt0 = (ch % CPB) * 4
    nc.vector.tensor_copy(
        g.v2[b][:, mt0:mt0 + 4, :, 0:64],
        vn[:].rearrange("p (t h e) -> p t h e", t=4, h=2))


def _attn_head(g, b, h):
    # m-tile outer over full n=2048; S tiles are [128, 1024] double-buffered
    # so S(mt+1) never waits on exp(mt); all four S matmuls are emitted
    # before the four U matmuls so the PE stays busy while ACT runs exp.
    nc = g.nc
    qt, kt, v2 = g.qt[b], g.kt[b], g.v2[b]
    hp = bass.ds(h * 64, 64)
    u = [g.pu.tile([128, 512], F32, tag=f"u{j}", name=f"u{j}")
         for j in range(NH)]
    for mt in range(MT_PER_B):
        mcols = bass.ds(mt * 128, 128)
        s0 = g.ps.tile([128, 1024], F32, tag="s", name="s0")
        nc.tensor.matmul(s0[:, 0:512], kt[hp, mcols], qt[hp, 0:512])
        nc.tensor.matmul(s0[:, 512:1024], kt[hp, mcols], qt[hp, 512:1024])
        s1 = g.ps.tile([128, 1024], F32, tag="s", name="s1")
        nc.tensor.matmul(s1[:, 0:512], kt[hp, mcols], qt[hp, 1024:1536])
        nc.tensor.matmul(s1[:, 512:1024], kt[hp, mcols], qt[hp, 1536:2048])
        e0 = g.esb.tile([128, 1024], BF16, tag="e", name="e0")
        nc.scalar.activation(
            e0[:], s0[:], mybir.ActivationFunctionType.Exp, scale=0.125)
        e1 = g.esb.tile([128, 1024], BF16, tag="e", name="e1")
        nc.scalar.activation(
            e1[:], s1[:], mybir.ActivationFunctionType.Exp, scale=0.125)
        mm = nc.tensor.matmul
        mm(u[0][:], v2[:, mt, h], e0[:, 0:512],
           start=(mt == 0), stop=(mt == MT_PER_B - 1), skip_group_check=True)
        mm(u[1][:], v2[:, mt, h], e0[:, 512:1024],
           start=(mt == 0), stop=(mt == MT_PER_B - 1), skip_group_check=True)
        mm(u[2][:], v2[:, mt, h], e1[:, 0:512],
           start=(mt == 0), stop=(mt == MT_PER_B - 1), skip_group_check=True)
        mm(u[3][:], v2[:, mt, h], e1[:, 512:1024],
           start=(mt == 0), stop=(mt == MT_PER_B - 1), skip_group_check=True)
    u_sb = g.usb.tile([65, N], BF16, tag="usb")
    for nh in range(NH):
        ncols = bass.ds(nh * 512, 512)
        nc.vector.tensor_copy(u_sb[:, ncols], u[nh][0:65, :])
    off = (b * HPC + h) * N
    nc.sync.dma_start(g.uout[:, off:off + N], u_sb[:])


_CACHED_NC = None


def kernel(x, c, Wq, bq, Wkv, bkv, q_gamma, k_gamma, _trace=False, _dbg=False):
    global LAST_EXEC_TIME_NS, LAST_RESULTS, _CACHED_NC, _LAST_IN_MAPS

    x = np.asarray(x, dtype=np.float32)
    c = np.asarray(c, dtype=np.float32)
    Wq = np.asarray(Wq, dtype=np.float32)
    bq = np.asarray(bq, dtype=np.float32)
    Wkv = np.asarray(Wkv, dtype=np.float32)
    bkv = np.asarray(bkv, dtype=np.float32)
    q_gamma = np.asarray(q_gamma, dtype=np.float32)
    k_gamma = np.asarray(k_gamma, dtype=np.float32)

    b, n, _ = x.shape
    xT = np.ascontiguousarray(x.reshape(ROWS, DIM).T.astype(NP_BF16))
    cT = np.ascontiguousarray(c.reshape(ROWS, DIM).T.astype(NP_BF16))

    g2 = q_gamma * k_gamma                      # [64]
    g2_2 = np.tile(g2, HPC)                     # [128]
    d2 = np.arange(DH)

    in_maps = []
    bv_locals = []
    for i in range(NC):
        h0 = i * HPC
        rows_q = np.concatenate(
            [h * DH + d2 for h in range(h0, h0 + HPC)])
        k_rows = np.concatenate(
            [h * 2 * DH + 2 * d2 for h in range(h0, h0 + HPC)])
        v_rows = k_rows + 1

        wq_t = np.ascontiguousarray(Wq[rows_q].T.astype(NP_BF16))
        wk_t = np.ascontiguousarray(
            (Wkv[k_rows] * g2_2[:, None]).T.astype(NP_BF16))
        wv_t = np.ascontiguousarray(Wkv[v_rows].T.astype(NP_BF16))
        bq_l = np.ascontiguousarray(bq[rows_q].reshape(128, 1))
        bk_l = np.ascontiguousarray((bkv[k_rows] * g2_2).reshape(128, 1))
        bv_locals.append(bkv[v_rows])

        gq_l = np.zeros((128, 2), dtype=np.float32)
        gk_l = np.zeros((128, 2), dtype=np.float32)
        for h in range(HPC):
            gq_l[h * DH:(h + 1) * DH, h] = 1.0
            gk_l[h * DH:(h + 1) * DH, h] = 1.0 / (g2 * g2)
        in_maps.append({
            "xT": xT, "cT": cT,
            "wq": wq_t, "wk": wk_t, "wv": wv_t,
            "bq": bq_l, "bk": bk_l,
            "gq": gq_l, "gk": gk_l,
        })

    _LAST_IN_MAPS = in_maps
    if _CACHED_NC is None:
        _CACHED_NC = build_bass(dbg=_dbg)
    nc = _CACHED_NC

    res = run_bass_kernel_spmd(
        nc, in_maps, core_ids=list(range(NC)), trace=_trace)
    LAST_EXEC_TIME_NS = res.exec_time_ns
    LAST_RESULTS = res

    # host-side finish: divide by the denominator (row 64), add bv, transpose
    full = np.empty((b, n, DIM), dtype=np.float32)
    for i in range(NC):
        u = np.asarray(res.results[i]["uout"], dtype=np.float32)  # [65, 8192]
        bv_l = bv_locals[i]
        for bb in range(B):
            for h in range(HPC):
                s = u[:, (bb * HPC + h) * N:(bb * HPC + h + 1) * N]
                o = (s[0:64] / s[64:65]).T + bv_l[h * DH:(h + 1) * DH]
                full[bb, :, i * 128 + h * DH: i * 128 + (h + 1) * DH] = o
    return full


# revision 8
# speedup vs baseline: 2.2529x; 1.5657x over previous
"""Cross-attention Bass kernel for Trainium2, 8 NeuronCores, head-sharded.

Reference semantics: q = RMSNorm_head(x@Wq.T+bq), kv = c@Wkv.T+bkv (k/v
interleaved), k = RMSNorm_head(k), out = softmax(q k^T/sqrt(dh)) v, merged
heads -> [b, n, dim].

Sharding: 16 heads over 8 cores (2 heads each). Each core reads full x, c
(pre-transposed and cast to bf16 on the host) and its weight slices; ships
un-normalized U.T tiles (with the softmax denominator in row 64) back to the
host, which does the final divide / bias-add / transpose in fp32.

Per-core pipeline (bf16 matmuls, fp32 PSUM):
  Projection phase (per 512-row chunk of xT/cT, 8 chunks over both batches):
    - plain DMA loads of xT/cT tiles [128k, 512seq] (host pre-transposed, so
      no on-device transposes of the activations)
    - W-stationary projections -> qT/kT/vT [head_dims, seq] in PSUM
    - RMSNorm in T layout: per-head sumsq via indicator-matmul (1/gamma^2
      folded for k; gamma_q*gamma_k folded into Wk/bk on host), sqrt on ACT,
      reciprocal on DVE, partition-broadcast with a K=2 expander matmul.
    - V transposed to natural [m, dh] (PE, 4 tiles/chunk) with a ones column
      at position 64 so the softmax denominator rides the AV matmul. V's bias
      is NOT applied on device: softmax(P) @ (V0+bv) = P@V0/den + bv, so the
      host adds bv after the division.
  Attention phase (per batch, per head, m-tile outer over full n=2048):
    S.T[m,:] = kt_tile.T @ qt (4 N=512 matmuls from one stationary load),
    exp(S.T/8) on ACT in one [128,2048] op (PSUM->SBUF bf16),
    U.T[nh] += V'.T @ expS.T into 4 PSUM accumulation banks.
    After 16 m-tiles: copy U.T rows 0..64 (64 dims + denominator) to SBUF
    bf16 and DMA to DRAM; host finishes.

Phases are separated (not interleaved) so attention can use all 8 PSUM banks;
the exp() on ACT overlaps the S/U matmuls on PE within the m-tile pipeline.
"""

import sys

sys.path.insert(0, "/opt/trn_rl_repo")

import numpy as np
import ml_dtypes
from contextlib import ExitStack

import concourse.bass as bass
import concourse.tile as tile
from concourse import bacc, mybir
from concourse.bass_utils import run_bass_kernel_spmd
from concourse.masks import make_identity

F32 = mybir.dt.float32
F32R = mybir.dt.float32r
BF16 = mybir.dt.bfloat16
NP_BF16 = ml_dtypes.bfloat16

DIM = 1024
H = 16
DH = 64
B = 2
N = 2048
ROWS = B * N            # 4096 flattened rows
NC = 8
HPC = H // NC           # 2 heads per core
EPS = 1.1920928955078125e-07

NKB = DIM // 128        # 8 k-tiles
CPB = N // 512          # 4 chunks of 512 rows per batch
NCH = ROWS // 512       # 8 chunks total
MT_PER_B = N // 128     # 16 m-tiles per batch
NH = N // 512           # 4 n-slices of 512 per batch

LAST_EXEC_TIME_NS = None
LAST_RESULTS = None
_LAST_IN_MAPS = None


class _Ctx:
    pass


def build_bass(dbg=False, reps=1):
    nc = bacc.Bacc("TRN2", target_bir_lowering=False, debug=False)
    g = _Ctx()
    g.nc = nc

    g.xT = nc.dram_tensor("xT", [DIM, ROWS], BF16, kind="ExternalInput")
    g.cT = nc.dram_tensor("cT", [DIM, ROWS], BF16, kind="ExternalInput")
    g.wq = nc.dram_tensor("wq", [DIM, 128], BF16, kind="ExternalInput")
    g.wk = nc.dram_tensor("wk", [DIM, 128], BF16, kind="ExternalInput")
    g.wv = nc.dram_tensor("wv", [DIM, 128], BF16, kind="ExternalInput")
    g.bq_d = nc.dram_tensor("bq", [128, 1], F32, kind="ExternalInput")
    g.bk_d = nc.dram_tensor("bk", [128, 1], F32, kind="ExternalInput")
    g.gq_d = nc.dram_tensor("gq", [128, 2], F32R, kind="ExternalInput")
    g.gk_d = nc.dram_tensor("gk", [128, 2], F32R, kind="ExternalInput")
    # U.T tiles: rows 0..63 = sum_m P*V0, row 64 = softmax denominator.
    # One [65, 2048] tile per (batch, head), packed along the free dim.
    g.uout = nc.dram_tensor("uout", [65, B * HPC * N], BF16,
                            kind="ExternalOutput")

    with tile.TileContext(nc) as tc, ExitStack() as ctx:
        g.tc = tc
        const = ctx.enter_context(tc.tile_pool(name="const", bufs=1))
        resid = ctx.enter_context(tc.tile_pool(name="resid", bufs=1))
        g.ld = ctx.enter_context(tc.tile_pool(name="ld", bufs=4))
        g.tmp = ctx.enter_context(tc.tile_pool(name="tmpA", bufs=2))
        g.small = ctx.enter_context(tc.tile_pool(name="small", bufs=2))
        g.esb = ctx.enter_context(tc.tile_pool(name="esb", bufs=3))
        g.usb = ctx.enter_context(tc.tile_pool(name="usb", bufs=2))

        ident_f = const.tile([128, 128], F32, tag="identf")
        make_identity(nc, ident_f[:])
        g.ident = const.tile([128, 128], F32R, tag="ident")
        nc.vector.tensor_copy(g.ident[:], ident_f[:])

        g.wq_sb = const.tile([128, NKB, 128], BF16, tag="wq")
        g.wk_sb = const.tile([128, NKB, 128], BF16, tag="wk")
        g.wv_sb = const.tile([128, NKB, 128], BF16, tag="wv")
        nc.scalar.dma_start(
            g.wq_sb[:], g.wq[:].rearrange("(kb p) c -> p kb c", p=128))
        nc.scalar.dma_start(
            g.wk_sb[:], g.wk[:].rearrange("(kb p) c -> p kb c", p=128))
        nc.scalar.dma_start(
            g.wv_sb[:], g.wv[:].rearrange("(kb p) c -> p kb c", p=128))
        g.bq_sb = const.tile([128, 1], F32, tag="bq")
        g.bk_sb = const.tile([128, 1], F32, tag="bk")
        g.gq_sb = const.tile([128, 2], F32R, tag="gq")
        g.gk_sb = const.tile([128, 2], F32R, tag="gk")
        nc.sync.dma_start(g.bq_sb[:], g.bq_d[:])
        nc.sync.dma_start(g.bk_sb[:], g.bk_d[:])
        nc.sync.dma_start(g.gq_sb[:], g.gq_d[:])
        nc.sync.dma_start(g.gk_sb[:], g.gk_d[:])
        g.eps_sb = const.tile([128, 1], F32, tag="eps")
        nc.gpsimd.memset(g.eps_sb[:], EPS)

        # expander: expand[x, y] = 1 iff y//64 == x  (rb[p] = rinv[p//64])
        expand_f = const.tile([2, 128], F32, tag="expand_f")
        nc.gpsimd.memset(expand_f[:], 0.0)
        nc.gpsimd.affine_select(
            out=expand_f[:], in_=expand_f[:],
            compare_op=mybir.AluOpType.is_ge, fill=1.0,
            base=-64, pattern=[[1, 128]], channel_multiplier=-64)
        nc.gpsimd.affine_select(
            out=expand_f[:], in_=expand_f[:],
            compare_op=mybir.AluOpType.is_ge, fill=0.0,
            base=0, pattern=[[1, 128]], channel_multiplier=-64)
        g.expand_b = const.tile([2, 128], F32R, tag="expand_b")
        nc.vector.tensor_copy(g.expand_b[:], expand_f[:])

        ones64 = const.tile([128, 64], F32, tag="ones64")
        nc.gpsimd.memset(ones64[:], 1.0)
        z1 = const.tile([128, 1], F32, tag="z1")
        nc.gpsimd.memset(z1[:], 0.0)

        # per-batch residents (bf16)
        g.qt = [resid.tile([128, N], BF16, tag=f"qt{b}", name=f"qt{b}")
                for b in range(B)]
        g.kt = [resid.tile([128, N], BF16, tag=f"kt{b}", name=f"kt{b}")
                for b in range(B)]
        g.v2 = [resid.tile([128, MT_PER_B, 2, 128], BF16, tag=f"v2{b}",
                           name=f"v2{b}")
                for b in range(B)]
        for b in range(B):
            nc.vector.tensor_copy(
                g.v2[b][:, :, :, 64:65],
                ones64[:, 0:MT_PER_B * 2].rearrange(
                    "p (a b c) -> p a b c", a=MT_PER_B, b=2))
            nc.vector.tensor_copy(
                g.v2[b][:, :, :, 65:128],
                z1[:].broadcast_to((128, MT_PER_B, 2, 63)))

        for _ in range(reps):
            # all projections first (both batches), then attention; the
            # phases are separated so attention can use all 8 PSUM banks
            with tc.tile_pool(name="ppsum", bufs=3, space="PSUM") as pp, \
                    tc.tile_pool(name="npsum", bufs=2, space="PSUM") as pn, \
                    tc.tile_pool(name="vpsum", bufs=2, space="PSUM") as pv:
                g.pp, g.pn, g.pv = pp, pn, pv
                for ch in range(NCH):
                    _proj_chunk(g, ch)
            with tc.tile_pool(name="spsum", bufs=3, space="PSUM") as ps, \
                    tc.tile_pool(name="upsum", bufs=1, space="PSUM") as pu:
                g.ps, g.pu = ps, pu
                for b in range(B):
                    _attn_batch(g, b)

        if dbg:
            qt_d = nc.dram_tensor("qt_dbg", [128, ROWS], F32,
                                  kind="ExternalOutput")
            kt_d = nc.dram_tensor("kt_dbg", [128, ROWS], F32,
                                  kind="ExternalOutput")
            v2_d = nc.dram_tensor("v2_dbg", [128, ROWS * 2], F32,
                                  kind="ExternalOutput")
            for b in range(B):
                qf = g.usb.tile([128, N], F32, tag="dbgq")
                nc.vector.tensor_copy(qf[:], g.qt[b][:])
                nc.sync.dma_start(qt_d[:, b * N:(b + 1) * N], qf[:])
                kf = g.usb.tile([128, N], F32, tag="dbgk")
                nc.vector.tensor_copy(kf[:], g.kt[b][:])
                nc.sync.dma_start(kt_d[:, b * N:(b + 1) * N], kf[:])
                vf = g.usb.tile([128, N * 2], F32, tag="dbgv")
                nc.vector.tensor_copy(
                    vf[:], g.v2[b][:].rearrange("p a b e -> p (a b e)"))
                nc.sync.dma_start(v2_d[:, b * N * 2:(b + 1) * N * 2], vf[:])

    nc.compile()
    return nc


def _norm_pre(g, lin_ps, bias_sb, name):
    """bias-add + square (DVE); returns (s_sb, sq) for the sumsq matmul."""
    nc = g.nc
    s_sb = g.tmp.tile([128, 512], F32, tag="lin", name=f"lin_{name}")
    nc.vector.tensor_scalar_add(s_sb[:], lin_ps[:], bias_sb[:])
    sq = g.tmp.tile([128, 512], F32R, tag="sq", name=f"sq_{name}")
    nc.vector.tensor_tensor(
        out=sq[:], in0=s_sb[:], in1=s_sb[:], op=mybir.AluOpType.mult)
    return s_sb, sq


def _norm_ss(g, g_sb, sq, name):
    """per-head sumsq via indicator matmul -> [2, 512] PSUM."""
    nc = g.nc
    ss = g.pn.tile([128, 512], F32, tag="nrm", name=f"ss_{name}")
    nc.tensor.matmul(ss[0:2, :], g_sb[:], sq[:])
    return ss


def _norm_rinv(g, ss, name):
    """sqrt on ACT, then single-instruction approx reciprocal on DVE
    (nc.vector.reciprocal is ~6 cpe iterative-divide, 3.3us per [2,512];
    the approx variant is ~51 ULP which is far below the bf16 noise)."""
    nc = g.nc
    rms = g.small.tile([2, 512], F32, tag="rms", name=f"rms_{name}")
    nc.scalar.activation(
        rms[:], ss[0:2, :], mybir.ActivationFunctionType.Sqrt,
        bias=g.eps_sb[0:2, :], scale=1.0 / DH)
    rinv = g.small.tile([2, 512], F32, tag="rinv", name=f"rinv_{name}")
    nc.vector.reciprocal_approx_fast(rinv[:], rms[:])
    rinv_r = g.small.tile([2, 512], F32R, tag="rinvr", name=f"rinvr_{name}")
    nc.vector.tensor_copy(rinv_r[:], rinv[:])
    return rinv_r


def _norm_fin(g, s_sb, rinv, dst_ap, name):
    """broadcast rinv down partitions (K=2 expander matmul) and scale."""
    nc = g.nc
    rb = g.pn.tile([128, 512], F32, tag="nrm", name=f"rb_{name}")
    nc.tensor.matmul(rb[:], g.expand_b[:], rinv[:])
    nc.vector.tensor_tensor(
        out=dst_ap, in0=s_sb[:], in1=rb[:], op=mybir.AluOpType.mult)


def _proj_chunk(g, ch):
    # emission order keeps the PE stream dense: the norm matmuls (ss, rb)
    # land between the q/k/v projection groups so their DVE/ACT inputs have
    # had time to complete, and the PE never waits on the norm chain.
    nc = g.nc
    b = ch // CPB
    cols = bass.ds((ch % CPB) * 512, 512)
    dcols = bass.ds(ch * 512, 512)

    xt = g.ld.tile([128, NKB, 512], BF16, tag="xt")
    for kb in range(NKB):
        nc.sync.dma_start(xt[:, kb], g.xT[kb * 128:(kb + 1) * 128, dcols])
    ct = g.ld.tile([128, NKB, 512], BF16, tag="ct")
    for kb in range(NKB):
        nc.scalar.dma_start(ct[:, kb], g.cT[kb * 128:(kb + 1) * 128, dcols])

    q_ps = g.pp.tile([128, 512], F32, tag="qkv", name="q_ps")
    for kb in range(NKB):
        nc.tensor.matmul(q_ps[:], g.wq_sb[:, kb], xt[:, kb],
                         start=(kb == 0), stop=(kb == NKB - 1))
    sq_s, sq_sq = _norm_pre(g, q_ps, g.bq_sb, "q")

    k_ps = g.pp.tile([128, 512], F32, tag="qkv", name="k_ps")
    for kb in range(NKB):
        nc.tensor.matmul(k_ps[:], g.wk_sb[:, kb], ct[:, kb],
                         start=(kb == 0), stop=(kb == NKB - 1))
    ss_q = _norm_ss(g, g.gq_sb, sq_sq, "q")
    rinv_q = _norm_rinv(g, ss_q, "q")
    sk_s, sk_sq = _norm_pre(g, k_ps, g.bk_sb, "k")

    v_ps = g.pp.tile([128, 512], F32, tag="qkv", name="v_ps")
    for kb in range(NKB):
        nc.tensor.matmul(v_ps[:], g.wv_sb[:, kb], ct[:, kb],
                         start=(kb == 0), stop=(kb == NKB - 1))
    ss_k = _norm_ss(g, g.gk_sb, sk_sq, "k")
    rinv_k = _norm_rinv(g, ss_k, "k")
    v_sb = g.tmp.tile([128, 512], F32R, tag="vsb")
    nc.vector.tensor_copy(v_sb[:], v_ps[:])

    _norm_fin(g, sq_s, rinv_q, g.qt[b][:, cols], "q")
    vn = g.pv.tile([128, 512], F32, tag="vn")
    for t in range(4):
        nc.tensor.transpose(
            vn[:, t * 128:(t + 1) * 128].bitcast(F32R),
            v_sb[:, t * 128:(t + 1) * 128],
            g.ident[:])
    _norm_fin(g, sk_s, rinv_k, g.kt[b][:, cols], "k")
    mt0 = (ch % CPB) * 4
    nc.vector.tensor_copy(
        g.v2[b][:, mt0:mt0 + 4, :, 0:64],
        vn[:].rearrange("p (t h e) -> p t h e", t=4, h=2))


def _attn_batch(g, b):
    # Both heads together: the two K=64 S matmuls target disjoint PE row
    # groups (head 0 lives on partitions 0..63, head 1 on 64..127), so the
    # hardware runs them CONCURRENTLY (row tiling) - S costs one matmul's
    # wall time for both heads. exp covers both heads in one [128,1024] op.
    # U (K=128) accumulates per head into separate PSUM banks over 16
    # m-tiles; rows 0..63 are sum(P*V0), row 64 the softmax denominator.
    nc = g.nc
    qt, kt, v2 = g.qt[b], g.kt[b], g.v2[b]
    for nch in range(NH):
        ncols = bass.ds(nch * 512, 512)
        uA = g.pu.tile([128, 512], F32, tag="uA", name="uA")
        uB = g.pu.tile([128, 512], F32, tag="uB", name="uB")
        for mt in range(MT_PER_B):
            mcols = bass.ds(mt * 128, 128)
            s = g.ps.tile([128, 1024], F32, tag="s", name="s")
            nc.tensor.matmul(s[:, 0:512], kt[0:64, mcols], qt[0:64, ncols])
            nc.tensor.matmul(s[:, 512:1024], kt[64:128, mcols],
                             qt[64:128, ncols])
            e = g.esb.tile([128, 1024], BF16, tag="e", name="e")
            nc.scalar.activation(
                e[:], s[:], mybir.ActivationFunctionType.Exp, scale=0.125)
            nc.tensor.matmul(uA[:], v2[:, mt, 0], e[:, 0:512],
                             start=(mt == 0), stop=(mt == MT_PER_B - 1),
                             skip_group_check=True)
            nc.tensor.matmul(uB[:], v2[:, mt, 1], e[:, 512:1024],
                             start=(mt == 0), stop=(mt == MT_PER_B - 1),
                             skip_group_check=True)
        u_sb = g.usb.tile([65, 1024], BF16, tag="usb")
        nc.vector.tensor_copy(u_sb[:, 0:512], uA[0:65, :])
        nc.vector.tensor_copy(u_sb[:, 512:1024], uB[0:65, :])
        nc.sync.dma_start(
            g.uout[:, b * HPC * N + nch * 512: b * HPC * N + nch * 512 + 512],
            u_sb[:, 0:512])
        nc.sync.dma_start(
            g.uout[:, (b * HPC + 1) * N + nch * 512:
                   (b * HPC + 1) * N + nch * 512 + 512],
            u_sb[:, 512:1024])


_CACHED_NC = None


def kernel(x, c, Wq, bq, Wkv, bkv, q_gamma, k_gamma, _trace=False, _dbg=False):
    global LAST_EXEC_TIME_NS, LAST_RESULTS, _CACHED_NC, _LAST_IN_MAPS

    x = np.asarray(x, dtype=np.float32)
    c = np.asarray(c, dtype=np.float32)
    Wq = np.asarray(Wq, dtype=np.float32)
    bq = np.asarray(bq, dtype=np.float32)
    Wkv = np.asarray(Wkv, dtype=np.float32)
    bkv = np.asarray(bkv, dtype=np.float32)
    q_gamma = np.asarray(q_gamma, dtype=np.float32)
    k_gamma = np.asarray(k_gamma, dtype=np.float32)

    b, n, _ = x.shape
    xT = np.ascontiguousarray(x.reshape(ROWS, DIM).T.astype(NP_BF16))
    cT = np.ascontiguousarray(c.reshape(ROWS, DIM).T.astype(NP_BF16))

    g2 = q_gamma * k_gamma                      # [64]
    g2_2 = np.tile(g2, HPC)                     # [128]
    d2 = np.arange(DH)

    in_maps = []
    bv_locals = []
    for i in range(NC):
        h0 = i * HPC
        rows_q = np.concatenate(
            [h * DH + d2 for h in range(h0, h0 + HPC)])
        k_rows = np.concatenate(
            [h * 2 * DH + 2 * d2 for h in range(h0, h0 + HPC)])
        v_rows = k_rows + 1

        wq_t = np.ascontiguousarray(Wq[rows_q].T.astype(NP_BF16))
        wk_t = np.ascontiguousarray(
            (Wkv[k_rows] * g2_2[:, None]).T.astype(NP_BF16))
        wv_t = np.ascontiguousarray(Wkv[v_rows].T.astype(NP_BF16))
        bq_l = np.ascontiguousarray(bq[rows_q].reshape(128, 1))
        bk_l = np.ascontiguousarray((bkv[k_rows] * g2_2).reshape(128, 1))
        bv_locals.append(bkv[v_rows])

        gq_l = np.zeros((128, 2), dtype=np.float32)
        gk_l = np.zeros((128, 2), dtype=np.float32)
        for h in range(HPC):
            gq_l[h * DH:(h + 1) * DH, h] = 1.0
            gk_l[h * DH:(h + 1) * DH, h] = 1.0 / (g2 * g2)
        in_maps.append({
            "xT": xT, "cT": cT,
            "wq": wq_t, "wk": wk_t, "wv": wv_t,
            "bq": bq_l, "bk": bk_l,
            "gq": gq_l, "gk": gk_l,
        })

    _LAST_IN_MAPS = in_maps
    if _CACHED_NC is None:
        _CACHED_NC = build_bass(dbg=_dbg)
    nc = _CACHED_NC

    res = run_bass_kernel_spmd(
        nc, in_maps, core_ids=list(range(NC)), trace=_trace)
    LAST_EXEC_TIME_NS = res.exec_time_ns
    LAST_RESULTS = res

    # host-side finish: divide by the denominator (row 64), add bv, transpose
    full = np.empty((b, n, DIM), dtype=np.float32)
    for i in range(NC):
        u = np.asarray(res.results[i]["uout"], dtype=np.float32)  # [65, 8192]
        bv_l = bv_locals[i]
        for bb in range(B):
            for h in range(HPC):
                s = u[:, (bb * HPC + h) * N:(bb * HPC + h + 1) * N]
                o = (s[0:64] / s[64:65]).T + bv_l[h * DH:(h + 1) * DH]
                full[bb, :, i * 128 + h * DH: i * 128 + (h + 1) * DH] = o
    return full
